# revision 1
# baseline (speedup 1.0000x reference)
"""KAN feed-forward on Trainium2 — Bass/Tile kernel, 8-core data-parallel.

Math transform: each KAN layer is
    y = silu(x) @ scale_base + einsum('nig,iog,io->no', B(x), coef, scale_sp)
with B the (G=5, K=3) uniform-grid B-spline basis (8 funcs/dim, knots
t_j = -2.2 + 0.4 j, j=0..11).  All 8 basis functions are integer shifts of the
cardinal cubic B-spline b3, and b3(t) = (1/6) sum_k (-1)^k C(4,k) relu(t-k)^3.
With u = 2.5 x + 5.5 clamped to [0, 11] (all basis functions vanish exactly at
both clamp points, so clamping is exact), the spline path becomes a dense
matmul over NM=11 truncated-power features per input dim:
    y_sp[n,o] = sum_{i,m} relu(u_ni - m)^3 * W[m,i,o],   m = 0..10
(the m=11 feature is identically zero on the clamped domain). W folds the
binomial stencil, scale_sp and coef on the host.  The silu base path rides the
same PSUM accumulation as extra K-tiles.

Per-core layout (512 tokens/core):
  L1: out1[o, tok] (+=) over 48 K-tiles (4 silu + 44 spline), lhsT = W1 tiles,
      rhs = feature tiles [128, 512] built from xT by ACT/DVE.
  L2: out2[tok, o] (natural) over 96 K-tiles, lhsT = feature tile slices,
      rhs = W2 tiles [128, 512].  L1's PSUM output [h, tok] is exactly the
      transposed layout L2's feature construction needs — no transposes.
"""

import math
import os
import sys
from contextlib import ExitStack

import numpy as np

for _p in ("/opt/trn_rl_repo",):
    if _p not in sys.path:
        sys.path.insert(0, _p)

# ---------------------------------------------------------------- constants
NG = 8  # G + K spline coefficients per edge
NM = 11  # truncated powers m = 0..10
D, H, O = 512, 1024, 512
NCORES = 8
NTOK = 4096
TOK = NTOK // NCORES  # 512 tokens per core
P = 128
UMAX = 11.0

L1_NK = 4 + NM * 4  # 48 K-tiles of 128 (4 base + 44 spline)
L2_NK = 8 + NM * 8  # 96 K-tiles of 128 (8 base + 88 spline)

# dtype toggles for matmul operands: "f32" | "f16" | "bf16".
# fp32 streams through the PE at 1/2-1/4 rate; fp16 keeps full rate with a
# 2^-12 mantissa (values here: |R| <= 1331, |W| ~ 0.5 -- well inside range).
W_MODE = os.environ.get("KAN_W_DT", "f32")
R_MODE = os.environ.get("KAN_R_DT", "f32")

# L2 sq-pass engine split: m values whose (u-m)^2 runs on ACT (Square), rest DVE
L2_SQ_ACT_M = {1, 2}

_BUILD_CACHE: dict = {}


def _np_wdt():
    if W_MODE == "bf16":
        import ml_dtypes

        return ml_dtypes.bfloat16
    if W_MODE == "f16":
        return np.float16
    return np.float32


# ---------------------------------------------------------------- host prep
def _stencil() -> np.ndarray:
    S = np.zeros((NM, NG), np.float64)
    for m in range(NM):
        k = m - np.arange(NG)
        for g in range(NG):
            kk = m - g
            if 0 <= kk <= 4:
                S[m, g] = ((-1.0) ** kk) * math.comb(4, kk) / 6.0
    return S


def _pack_w1(coef1, scale_sp1, scale_base1) -> np.ndarray:
    """-> (48, 128, 1024): k-tile, rows(K-slice), cols (ob*128+c) of hidden."""
    S = _stencil()
    A = coef1.astype(np.float64) * scale_sp1.astype(np.float64)[:, :, None]
    W1s = np.einsum("mg,iog->mio", S, A)  # (11, 512, 1024)
    w1 = np.empty((L1_NK, P, H), np.float32)
    for ib in range(4):
        w1[ib] = scale_base1[ib * P : (ib + 1) * P]
    for m in range(NM):
        for ib in range(4):
            w1[4 + m * 4 + ib] = W1s[m, ib * P : (ib + 1) * P]
    return np.ascontiguousarray(w1.astype(_np_wdt()))


def _pack_w2(coef2, scale_sp2, scale_base2) -> np.ndarray:
    """-> (96, 128, 512): K-tile rows x output cols."""
    S = _stencil()
    A = coef2.astype(np.float64) * scale_sp2.astype(np.float64)[:, :, None]
    W2s = np.einsum("mg,iog->mio", S, A)  # (11, 1024, 512)
    w2 = np.empty((L2_NK, P, O), np.float32)
    for j in range(8):
        w2[j] = scale_base2[j * P : (j + 1) * P]
    for m in range(NM):
        for j in range(8):
            w2[8 + m * 8 + j] = W2s[m, j * P : (j + 1) * P]
    return np.ascontiguousarray(w2.astype(_np_wdt()))


# ---------------------------------------------------------------- bass build
def _emit_features(nc, tmp, rp, u_tile, m, r_dt, sq_on_act, name, bias_ap):
    """Emit ops computing r = relu(u - m)^3 as a [P, free] tile; returns AP."""
    import concourse.mybir as mybir

    AF = mybir.ActivationFunctionType
    free = u_tile.shape[-1]
    if m == 0:
        s_ap = u_tile  # u >= 0 already
    else:
        s = tmp.tile([P, free], mybir.dt.float32, tag="s", name=f"s{name}m{m}")
        nc.scalar.activation(s, u_tile, AF.Relu, bias=bias_ap(float(-m)))
        s_ap = s
    sq = tmp.tile([P, free], mybir.dt.float32, tag="q", name=f"q{name}m{m}")
    if sq_on_act:
        nc.scalar.activation(sq, u_tile, AF.Square, bias=bias_ap(float(-m)))
    else:
        nc.vector.tensor_mul(sq, s_ap, s_ap)
    r = rp.tile([P, free], r_dt, tag="r", name=f"r{name}m{m}")
    nc.vector.tensor_mul(r, sq, s_ap)
    return r


def _build_kernel():
    """Build + compile the Bass program once; cached per process."""
    if "nc" in _BUILD_CACHE:
        return _BUILD_CACHE["nc"]

    import concourse.mybir as mybir
    import concourse.tile as tile
    from concourse import bacc

    AF = mybir.ActivationFunctionType
    F32 = mybir.dt.float32
    _dt = {"f32": F32, "f16": mybir.dt.float16, "bf16": mybir.dt.bfloat16}
    WDT = _dt[W_MODE]
    RDT = _dt[R_MODE]

    nc = bacc.Bacc("TRN2", target_bir_lowering=False, debug=False, num_devices=NCORES)

    xT = nc.dram_tensor("xT", (D, TOK), F32, kind="ExternalInput").ap()
    w1 = nc.dram_tensor("w1", (L1_NK, P, H), WDT, kind="ExternalInput").ap()
    w2 = nc.dram_tensor("w2", (L2_NK, P, O), WDT, kind="ExternalInput").ap()
    out = nc.dram_tensor("out", (TOK, O), F32, kind="ExternalOutput").ap()

    with tile.TileContext(nc) as tc, ExitStack() as ctx:
        persist = ctx.enter_context(tc.tile_pool(name="persist", bufs=1))
        tmp = ctx.enter_context(tc.tile_pool(name="tmp", bufs=3))
        rp = ctx.enter_context(tc.tile_pool(name="rp", bufs=4))
        w1p = ctx.enter_context(tc.tile_pool(name="w1p", bufs=4))
        w2p = ctx.enter_context(tc.tile_pool(name="w2p", bufs=6))
        outp = ctx.enter_context(tc.tile_pool(name="outp", bufs=4))
        psum = ctx.enter_context(tc.tile_pool(name="psum", bufs=1, space="PSUM"))

        _bias_cache: dict = {}

        def bias_ap(val: float):
            if val not in _bias_cache:
                t = persist.tile([P, 1], F32, tag=f"bias{len(_bias_cache)}",
                                 name=f"bias_{len(_bias_cache)}")
                nc.vector.memset(t, val)
                _bias_cache[val] = t
            return _bias_cache[val]

        # ---- L1 inputs: xT tiles + activations --------------------------
        xt = []
        for ib in range(4):
            t = persist.tile([P, TOK], F32, tag="xt", bufs=2, name=f"xt{ib}")
            nc.sync.dma_start(out=t, in_=xT[ib * P : (ib + 1) * P, :])
            xt.append(t)

        u1, si1 = [], []
        for ib in range(4):
            t1 = tmp.tile([P, TOK], F32, tag="t1", name=f"t1_{ib}")
            nc.scalar.activation(t1, xt[ib], AF.Relu, bias=bias_ap(5.5), scale=2.5)
            u = persist.tile([P, TOK], F32, tag=f"u1{ib}", name=f"u1_{ib}")
            nc.vector.tensor_scalar_min(u, t1, UMAX)
            u1.append(u)
            s = persist.tile([P, TOK], RDT, tag=f"si1{ib}", name=f"si1_{ib}")
            nc.scalar.activation(s, xt[ib], AF.Silu, bias=bias_ap(0.0))
            si1.append(s)

        # ---- L1 matmuls: out1[o_blk, tok] accumulated over 48 K-tiles ---
        pb = [
            psum.tile([P, TOK], F32, tag=f"p{ob}", name=f"p{ob}") for ob in range(8)
        ]

        def l1_block(k, rhs_ap):
            wt = w1p.tile([P, H], WDT, tag="w1k", name=f"w1k{k}")
            nc.sync.dma_start(out=wt, in_=w1[k])
            last = k == L1_NK - 1
            for ob in range(8):
                nc.tensor.matmul(
                    pb[ob],
                    wt[:, ob * P : (ob + 1) * P],
                    rhs_ap,
                    start=(k == 0),
                    stop=last,
                )

        for k in range(4):  # silu base path
            l1_block(k, si1[k])
        for m in range(NM):
            for ib in range(4):
                r = _emit_features(nc, tmp, rp, u1[ib], m, RDT, sq_on_act=True,
                                   name=f"a{ib}", bias_ap=bias_ap)
                l1_block(4 + m * 4 + ib, r)

        # ---- boundary: h = out1 lives in PSUM [h_blk, tok]; derive L2 feats
        u2, si2 = [], []
        for j in range(8):
            t1b = tmp.tile([P, TOK], F32, tag="t1", name=f"t1b_{j}")
            nc.scalar.activation(t1b, pb[j], AF.Relu, bias=bias_ap(5.5), scale=2.5)
            s = persist.tile([P, TOK], RDT, tag=f"si2{j}", name=f"si2_{j}")
            nc.scalar.activation(s, pb[j], AF.Silu, bias=bias_ap(0.0))
            si2.append(s)
            u = persist.tile([P, TOK], F32, tag=f"u2{j}", name=f"u2_{j}")
            nc.vector.tensor_scalar_min(u, t1b, UMAX)
            u2.append(u)

        # ---- L2 matmuls: out2[tok_blk, o] over 96 K-tiles ---------------
        qb = [
            psum.tile([P, O], F32, tag=f"p{tb}", name=f"q{tb}") for tb in range(4)
        ]

        def l2_block(k, lhsT_tile):
            wt = w2p.tile([P, O], WDT, tag="w2k", name=f"w2k{k}")
            nc.sync.dma_start(out=wt, in_=w2[k])
            last = k == L2_NK - 1
            for tb in range(4):
                nc.tensor.matmul(
                    qb[tb],
                    lhsT_tile[:, tb * P : (tb + 1) * P],
                    wt,
                    start=(k == 0),
                    stop=last,
                )

        for k in range(8):  # silu base path
            l2_block(k, si2[k])
        for m in range(NM):
            for j in range(8):
                r = _emit_features(nc, tmp, rp, u2[j], m, RDT,
                                   sq_on_act=(m in L2_SQ_ACT_M), name=f"b{j}", bias_ap=bias_ap)
                l2_block(8 + m * 8 + j, r)

        # ---- store ------------------------------------------------------
        for tb in range(4):
            ot = outp.tile([P, O], F32, tag="ot", name=f"ot{tb}")
            nc.vector.tensor_copy(ot, qb[tb])
            nc.sync.dma_start(out=out[tb * P : (tb + 1) * P, :], in_=ot)

    nc.compile()
    _BUILD_CACHE["nc"] = nc
    return nc


# ---------------------------------------------------------------- entry
def kernel(x, coef1, scale_base1, scale_sp1, coef2, scale_base2, scale_sp2,
           _want_trace=False):
    from concourse.bass_utils import run_bass_kernel_spmd

    wdt = _np_wdt()
    x_flat = np.asarray(x, np.float32).reshape(NTOK, D)
    w1 = _pack_w1(np.asarray(coef1), np.asarray(scale_sp1), np.asarray(scale_base1))
    w2 = _pack_w2(np.asarray(coef2), np.asarray(scale_sp2), np.asarray(scale_base2))

    nc = _build_kernel()

    in_maps = []
    for c in range(NCORES):
        xs = x_flat[c * TOK : (c + 1) * TOK]  # (TOK, D)
        in_maps.append(
            {
                "xT": np.ascontiguousarray(xs.T),
                "w1": w1,
                "w2": w2,
            }
        )

    res = run_bass_kernel_spmd(
        nc, in_maps, core_ids=list(range(NCORES)), trace=_want_trace
    )
    outs = [res.results[c]["out"] for c in range(NCORES)]
    full = np.concatenate(outs, axis=0).reshape(x.shape[0], x.shape[1], O)
    if _want_trace:
        kernel._last_results = res  # stash for test harness profiling
    return full.astype(np.float32)



# revision 11
# speedup vs baseline: 3.2358x; 3.2358x over previous
"""KAN feed-forward on Trainium2 — Bass/Tile kernel, 8-core data-parallel.

Math transform: each KAN layer is
    y = silu(x) @ scale_base + einsum('nig,iog,io->no', B(x), coef, scale_sp)
with B the (G=5, K=3) uniform-grid B-spline basis: B_g(x) = b3(u - g),
u = 2.5 x + 5.5, g = 0..7, b3 the cardinal cubic B-spline on [0, 4].

Unlike the truncated-power expansion (features up to 11^3 = 1331 with massive
cancellation against the output scale ~12, which needs fp32 matmuls), the
B-spline basis is evaluated DIRECTLY on ACT/DVE in bounded arithmetic:
    x_g = min(u, 2g+4-u)        # tent coordinate; <0 outside support
    p   = relu(x_g - g)         # in [0, 2]
    q   = relu(x_g - g - 1)     # in [0, 1]
    Bt  = 4 q^3 - p^3 = -6 b3(u - g)   # in [-4, 0]
All intermediates are O(1) so fp16 feature/weight matmuls are accurate
(rel err ~2e-3 end to end), running the PE at full rate (4x over fp32),
with only 9 K-feature-groups (silu + 8 B) instead of 12.

Per-core layout (512 tokens/core, data-parallel over tokens):
  L1: out1[o_blk, tok] accumulated over 36 K-tiles (4 silu + 8 g x 4 iblk).
      Elementwise feature build batched at free-dim 2048 (all 4 iblks).
  L2: out2[tok_blk, o] over 72 K-tiles (8 silu + 8 g x 8 jblk); features
      batched at free-dim 4096.  L1's PSUM output [h, tok] is exactly the
      layout L2's feature construction needs — no transposes anywhere.
"""

import sys
from contextlib import ExitStack

import numpy as np

for _p in ("/opt/trn_rl_repo",):
    if _p not in sys.path:
        sys.path.insert(0, _p)

# ---------------------------------------------------------------- constants
NB = 8  # B-spline basis functions per input dim
D, H, O = 512, 1024, 512
NCORES = 8
NTOK = 4096
TOK = NTOK // NCORES  # 512 tokens per core
P = 128

L1_NK = 4 + NB * 4  # 36 K-tiles of 128
L2_NK = 8 + NB * 8  # 72 K-tiles of 128

_BUILD_CACHE: dict = {}


# ---------------------------------------------------------------- host prep
def _pack_w1(coef1, scale_sp1, scale_base1) -> np.ndarray:
    """-> (36, 128, 1024) f16: K-tile rows x hidden cols."""
    Wg = -(coef1.astype(np.float64) * scale_sp1.astype(np.float64)[:, :, None]) / 6.0
    w1 = np.empty((L1_NK, P, H), np.float16)
    for ib in range(4):
        w1[ib] = scale_base1[ib * P : (ib + 1) * P]
    for g in range(NB):
        for ib in range(4):
            w1[4 + g * 4 + ib] = Wg[ib * P : (ib + 1) * P, :, g]
    return np.ascontiguousarray(w1)


def _pack_w2(coef2, scale_sp2, scale_base2) -> np.ndarray:
    """-> (72, 128, 512) f16: K-tile rows x output cols."""
    Wg = -(coef2.astype(np.float64) * scale_sp2.astype(np.float64)[:, :, None]) / 6.0
    w2 = np.empty((L2_NK, P, O), np.float16)
    for j in range(8):
        w2[j] = scale_base2[j * P : (j + 1) * P]
    for g in range(NB):
        for j in range(8):
            w2[8 + g * 8 + j] = Wg[j * P : (j + 1) * P, :, g]
    return np.ascontiguousarray(w2)


# ---------------------------------------------------------------- bass build
def _build_kernel():
    """Build + compile the Bass program once; cached per process."""
    if "nc" in _BUILD_CACHE:
        return _BUILD_CACHE["nc"]

    import concourse.mybir as mybir
    import concourse.tile as tile
    from concourse import bacc

    AF = mybir.ActivationFunctionType
    OP = mybir.AluOpType
    F32 = mybir.dt.float32
    F16 = mybir.dt.float16

    nc = bacc.Bacc("TRN2", target_bir_lowering=False, debug=False, num_devices=NCORES)

    xT = nc.dram_tensor("xT", (D, TOK), F32, kind="ExternalInput").ap()
    w1 = nc.dram_tensor("w1", (L1_NK, P, H), F16, kind="ExternalInput").ap()
    w2 = nc.dram_tensor("w2", (L2_NK, P, O), F16, kind="ExternalInput").ap()
    out = nc.dram_tensor("out", (TOK, O), F32, kind="ExternalOutput").ap()

    with tile.TileContext(nc) as tc, ExitStack() as ctx:
        persist = ctx.enter_context(tc.tile_pool(name="persist", bufs=1))
        tmp1 = ctx.enter_context(tc.tile_pool(name="tmp1", bufs=2))
        bt1 = ctx.enter_context(tc.tile_pool(name="bt1", bufs=3))
        tmp2 = ctx.enter_context(tc.tile_pool(name="tmp2", bufs=2))
        bt2 = ctx.enter_context(tc.tile_pool(name="bt2", bufs=3))
        w1p = ctx.enter_context(tc.tile_pool(name="w1p", bufs=4))
        w2p = ctx.enter_context(tc.tile_pool(name="w2p", bufs=5))
        outp = ctx.enter_context(tc.tile_pool(name="outp", bufs=2))
        psum = ctx.enter_context(tc.tile_pool(name="psum", bufs=1, space="PSUM"))

        F1 = 4 * TOK   # L1 elementwise free dim (4 i-blocks batched)
        F2 = 8 * TOK   # L2 elementwise free dim (8 h-blocks batched)

        _bias_cache: dict = {}

        def bias_ap(val: float):
            if val not in _bias_cache:
                t = persist.tile([P, 1], F32, tag=f"bias{len(_bias_cache)}",
                                 name=f"bias_{len(_bias_cache)}")
                nc.vector.memset(t, val)
                _bias_cache[val] = t
            return _bias_cache[val]

        # ---- L1 prep: x -> u, silu -----------------------------------
        xb = persist.tile([P, F1], F32, tag="xb", name="xb")
        for ib in range(4):
            nc.sync.dma_start(
                out=xb[:, ib * TOK : (ib + 1) * TOK],
                in_=xT[ib * P : (ib + 1) * P, :],
            )
        # u = relu(2.5x + 5.5): clamp-at-0 is exact (all basis funcs vanish)
        u1 = persist.tile([P, F1], F16, tag="u1", name="u1")
        nc.scalar.activation(u1, xb, AF.Relu, bias=bias_ap(5.5), scale=2.5)
        r1 = persist.tile([P, F1], F16, tag="r1", name="r1")
        nc.vector.tensor_scalar_mul(r1, u1, -1.0)
        si1 = persist.tile([P, F1], F16, tag="si1", name="si1")
        nc.scalar.activation(si1, xb, AF.Silu)

        # ---- L1 matmuls: out1[o_blk, tok] over 36 K-tiles -------------
        pb = [psum.tile([P, TOK], F32, tag=f"p{ob}", name=f"p{ob}") for ob in range(8)]

        def l1_block(k, rhs_ap):
            wt = w1p.tile([P, H], F16, tag="w1k", name=f"w1k{k}")
            nc.sync.dma_start(out=wt, in_=w1[k])
            last = k == L1_NK - 1
            for ob in range(8):
                nc.tensor.matmul(
                    pb[ob],
                    wt[:, ob * P : (ob + 1) * P],
                    rhs_ap,
                    start=(k == 0),
                    stop=last,
                )

        def emit_bspline(g, u, r, free, tmp, btp, name):
            """Bt_g = 4q^3 - p^3 = -6*b3(u-g), all f16, bounded."""
            a = tmp.tile([P, free], F16, tag="a", bufs=1, name=f"a{name}{g}")
            nc.vector.tensor_scalar_add(a, r, float(2 * g + 4))  # 2g+4 - u
            xg = tmp.tile([P, free], F16, tag="xg", name=f"xg{name}{g}")
            nc.vector.tensor_tensor(xg, a, u, OP.min)            # tent coord
            p = tmp.tile([P, free], F16, tag="p", name=f"p{name}{g}")
            nc.vector.tensor_scalar(p, xg, float(g), 0.0, OP.subtract, OP.max)
            q = tmp.tile([P, free], F16, tag="q", name=f"q{name}{g}")
            nc.vector.tensor_scalar(q, xg, float(g + 1), 0.0, OP.subtract, OP.max)
            p2 = tmp.tile([P, free], F16, tag="p2", name=f"p2{name}{g}")
            nc.scalar.activation(p2, p, AF.Square)
            q24 = tmp.tile([P, free], F16, tag="q24", name=f"q24{name}{g}")
            nc.scalar.activation(q24, q, AF.Square, scale=2.0)  # (2q)^2 = 4q^2
            p3 = tmp.tile([P, free], F16, tag="p3", bufs=1, name=f"p3{name}{g}")
            nc.vector.tensor_mul(p3, p2, p)
            q34 = tmp.tile([P, free], F16, tag="q34", bufs=1, name=f"q34{name}{g}")
            nc.vector.tensor_mul(q34, q24, q)
            bt = btp.tile([P, free], F16, tag="bt", name=f"bt{name}{g}")
            nc.vector.tensor_sub(bt, q34, p3)
            return bt

        for ib in range(4):  # silu base path
            l1_block(ib, si1[:, ib * TOK : (ib + 1) * TOK])
        for g in range(NB):
            bt = emit_bspline(g, u1, r1, F1, tmp1, bt1, "a")
            for ib in range(4):
                l1_block(4 + g * 4 + ib, bt[:, ib * TOK : (ib + 1) * TOK])

        # ---- boundary: h in PSUM [h_blk, tok]; derive L2 u/silu -------
        u2 = persist.tile([P, F2], F16, tag="u2", name="u2")
        si2 = persist.tile([P, F2], F16, tag="si2", name="si2")
        for j in range(8):
            nc.scalar.activation(
                u2[:, j * TOK : (j + 1) * TOK], pb[j], AF.Relu, bias=bias_ap(5.5), scale=2.5
            )
            nc.scalar.activation(si2[:, j * TOK : (j + 1) * TOK], pb[j], AF.Silu)
        r2 = persist.tile([P, F2], F16, tag="r2", name="r2")
        nc.vector.tensor_scalar_mul(r2, u2, -1.0)

        # ---- L2 matmuls: out2[tok_blk, o] over 72 K-tiles -------------
        qb = [psum.tile([P, O], F32, tag=f"p{tb}", name=f"q{tb}") for tb in range(4)]

        def l2_block(k, lhsT_tile):
            wt = w2p.tile([P, O], F16, tag="w2k", name=f"w2k{k}")
            nc.sync.dma_start(out=wt, in_=w2[k])
            last = k == L2_NK - 1
            for tb in range(4):
                nc.tensor.matmul(
                    qb[tb],
                    lhsT_tile[:, tb * P : (tb + 1) * P],
                    wt,
                    start=(k == 0),
                    stop=last,
                )

        for j in range(8):  # silu base path
            l2_block(j, si2[:, j * TOK : (j + 1) * TOK])
        HF = F2 // 2  # L2 features built in two halves (SBUF pressure)
        for hf in range(2):
            sl = slice(hf * HF, (hf + 1) * HF)
            for g in range(NB):
                bt = emit_bspline(g, u2[:, sl], r2[:, sl], HF, tmp2, bt2, f"b{hf}")
                for jj in range(4):
                    j = hf * 4 + jj
                    l2_block(8 + g * 8 + j, bt[:, jj * TOK : (jj + 1) * TOK])

        # ---- store ----------------------------------------------------
        for tb in range(4):
            ot = outp.tile([P, O], F32, tag="ot", name=f"ot{tb}")
            nc.vector.tensor_copy(ot, qb[tb])
            nc.sync.dma_start(out=out[tb * P : (tb + 1) * P, :], in_=ot)

    nc.compile()
    _BUILD_CACHE["nc"] = nc
    return nc


# ---------------------------------------------------------------- entry
def kernel(x, coef1, scale_base1, scale_sp1, coef2, scale_base2, scale_sp2,
           _want_trace=False):
    from concourse.bass_utils import run_bass_kernel_spmd

    x_flat = np.asarray(x, np.float32).reshape(NTOK, D)
    w1 = _pack_w1(np.asarray(coef1), np.asarray(scale_sp1), np.asarray(scale_base1))
    w2 = _pack_w2(np.asarray(coef2), np.asarray(scale_sp2), np.asarray(scale_base2))

    nc = _build_kernel()

    in_maps = []
    for c in range(NCORES):
        xs = x_flat[c * TOK : (c + 1) * TOK]  # (TOK, D)
        in_maps.append(
            {
                "xT": np.ascontiguousarray(xs.T),
                "w1": w1,
                "w2": w2,
            }
        )

    res = run_bass_kernel_spmd(
        nc, in_maps, core_ids=list(range(NCORES)), trace=_want_trace
    )
    outs = [res.results[c]["out"] for c in range(NCORES)]
    full = np.concatenate(outs, axis=0).reshape(x.shape[0], x.shape[1], O)
    if _want_trace:
        kernel._last_results = res  # stash for test harness profiling
    return full.astype(np.float32)


# revision 12
# speedup vs baseline: 3.2396x; 1.0012x over previous
"""KAN feed-forward on Trainium2 — Bass/Tile kernel, 8-core data-parallel.

Math transform: each KAN layer is
    y = silu(x) @ scale_base + einsum('nig,iog,io->no', B(x), coef, scale_sp)
with B the (G=5, K=3) uniform-grid B-spline basis: B_g(x) = b3(u - g),
u = 2.5 x + 5.5, g = 0..7, b3 the cardinal cubic B-spline on [0, 4].

Unlike the truncated-power expansion (features up to 11^3 = 1331 with massive
cancellation against the output scale ~12, which needs fp32 matmuls), the
B-spline basis is evaluated DIRECTLY on ACT/DVE in bounded arithmetic:
    x_g = min(u, 2g+4-u)        # tent coordinate; <0 outside support
    p   = relu(x_g - g)         # in [0, 2]
    q   = relu(x_g - g - 1)     # in [0, 1]
    Bt  = 4 q^3 - p^3 = -6 b3(u - g)   # in [-4, 0]
All intermediates are O(1) so fp16 feature/weight matmuls are accurate
(rel err ~2e-3 end to end), running the PE at full rate (4x over fp32),
with only 9 K-feature-groups (silu + 8 B) instead of 12.

Per-core layout (512 tokens/core, data-parallel over tokens):
  L1: out1[o_blk, tok] accumulated over 36 K-tiles (4 silu + 8 g x 4 iblk).
      Elementwise feature build batched at free-dim 2048 (all 4 iblks).
  L2: out2[tok_blk, o] over 72 K-tiles (8 silu + 8 g x 8 jblk); features
      batched at free-dim 4096.  L1's PSUM output [h, tok] is exactly the
      layout L2's feature construction needs — no transposes anywhere.
"""

import sys
from contextlib import ExitStack

import numpy as np

for _p in ("/opt/trn_rl_repo",):
    if _p not in sys.path:
        sys.path.insert(0, _p)

# ---------------------------------------------------------------- constants
NB = 8  # B-spline basis functions per input dim
D, H, O = 512, 1024, 512
NCORES = 8
NTOK = 4096
TOK = NTOK // NCORES  # 512 tokens per core
P = 128

L1_NK = 4 + NB * 4  # 36 K-tiles of 128
L2_NK = 8 + NB * 8  # 72 K-tiles of 128

_BUILD_CACHE: dict = {}


# ---------------------------------------------------------------- host prep
def _pack_w1(coef1, scale_sp1, scale_base1) -> np.ndarray:
    """-> (36, 128, 1024) f16: K-tile rows x hidden cols."""
    Wg = -(coef1.astype(np.float64) * scale_sp1.astype(np.float64)[:, :, None]) / 6.0
    w1 = np.empty((L1_NK, P, H), np.float16)
    for ib in range(4):
        w1[ib] = scale_base1[ib * P : (ib + 1) * P]
    for g in range(NB):
        for ib in range(4):
            w1[4 + g * 4 + ib] = Wg[ib * P : (ib + 1) * P, :, g]
    return np.ascontiguousarray(w1)


def _pack_w2(coef2, scale_sp2, scale_base2) -> np.ndarray:
    """-> (72, 128, 512) f16: K-tile rows x output cols."""
    Wg = -(coef2.astype(np.float64) * scale_sp2.astype(np.float64)[:, :, None]) / 6.0
    w2 = np.empty((L2_NK, P, O), np.float16)
    for j in range(8):
        w2[j] = scale_base2[j * P : (j + 1) * P]
    for g in range(NB):
        for j in range(8):
            w2[8 + g * 8 + j] = Wg[j * P : (j + 1) * P, :, g]
    return np.ascontiguousarray(w2)


# ---------------------------------------------------------------- bass build
def _build_kernel():
    """Build + compile the Bass program once; cached per process."""
    if "nc" in _BUILD_CACHE:
        return _BUILD_CACHE["nc"]

    import concourse.mybir as mybir
    import concourse.tile as tile
    from concourse import bacc

    AF = mybir.ActivationFunctionType
    OP = mybir.AluOpType
    F32 = mybir.dt.float32
    F16 = mybir.dt.float16

    nc = bacc.Bacc("TRN2", target_bir_lowering=False, debug=False, num_devices=NCORES)

    xT = nc.dram_tensor("xT", (D, TOK), F32, kind="ExternalInput").ap()
    w1 = nc.dram_tensor("w1", (L1_NK, P, H), F16, kind="ExternalInput").ap()
    w2 = nc.dram_tensor("w2", (L2_NK, P, O), F16, kind="ExternalInput").ap()
    out = nc.dram_tensor("out", (TOK, O), F32, kind="ExternalOutput").ap()

    with tile.TileContext(nc) as tc, ExitStack() as ctx:
        persist = ctx.enter_context(tc.tile_pool(name="persist", bufs=1))
        tmp1 = ctx.enter_context(tc.tile_pool(name="tmp1", bufs=2))
        bt1 = ctx.enter_context(tc.tile_pool(name="bt1", bufs=3))
        tmp2 = ctx.enter_context(tc.tile_pool(name="tmp2", bufs=2))
        bt2 = ctx.enter_context(tc.tile_pool(name="bt2", bufs=3))
        w1p = ctx.enter_context(tc.tile_pool(name="w1p", bufs=4))
        w2p = ctx.enter_context(tc.tile_pool(name="w2p", bufs=5))
        outp = ctx.enter_context(tc.tile_pool(name="outp", bufs=2))
        psum = ctx.enter_context(tc.tile_pool(name="psum", bufs=1, space="PSUM"))

        F1 = 4 * TOK   # L1 elementwise free dim (4 i-blocks batched)
        F2 = 8 * TOK   # L2 elementwise free dim (8 h-blocks batched)

        _bias_cache: dict = {}

        def bias_ap(val: float):
            if val not in _bias_cache:
                t = persist.tile([P, 1], F32, tag=f"bias{len(_bias_cache)}",
                                 name=f"bias_{len(_bias_cache)}")
                nc.vector.memset(t, val)
                _bias_cache[val] = t
            return _bias_cache[val]

        # ---- L1 prep: x -> u, silu -----------------------------------
        xb = persist.tile([P, F1], F32, tag="xb", name="xb")
        for ib in range(4):
            nc.sync.dma_start(
                out=xb[:, ib * TOK : (ib + 1) * TOK],
                in_=xT[ib * P : (ib + 1) * P, :],
            )
        # u = relu(2.5x + 5.5): clamp-at-0 is exact (all basis funcs vanish)
        u1 = persist.tile([P, F1], F16, tag="u1", name="u1")
        nc.scalar.activation(u1, xb, AF.Relu, bias=bias_ap(5.5), scale=2.5)
        r1 = persist.tile([P, F1], F16, tag="r1", name="r1")
        nc.vector.tensor_scalar_mul(r1, u1, -1.0)
        si1 = persist.tile([P, F1], F16, tag="si1", name="si1")
        nc.scalar.activation(si1, xb, AF.Silu)

        # ---- L1 matmuls: out1[o_blk, tok] over 36 K-tiles -------------
        pb = [psum.tile([P, TOK], F32, tag=f"p{ob}", name=f"p{ob}") for ob in range(8)]

        def l1_block(k, rhs_ap):
            wt = w1p.tile([P, H], F16, tag="w1k", name=f"w1k{k}")
            nc.sync.dma_start(out=wt, in_=w1[k])
            last = k == L1_NK - 1
            for ob in range(8):
                nc.tensor.matmul(
                    pb[ob],
                    wt[:, ob * P : (ob + 1) * P],
                    rhs_ap,
                    start=(k == 0),
                    stop=last,
                )

        def emit_bspline(g, u, r, free, tmp, btp, name):
            """Bt_g = 4q^3 - p^3 = -6*b3(u-g), all f16, bounded."""
            a = tmp.tile([P, free], F16, tag="a", bufs=1, name=f"a{name}{g}")
            nc.vector.tensor_scalar_add(a, r, float(2 * g + 4))  # 2g+4 - u
            xg = tmp.tile([P, free], F16, tag="xg", name=f"xg{name}{g}")
            nc.vector.tensor_tensor(xg, a, u, OP.min)            # tent coord
            p = tmp.tile([P, free], F16, tag="p", name=f"p{name}{g}")
            nc.vector.tensor_scalar(p, xg, float(g), 0.0, OP.subtract, OP.max)
            q = tmp.tile([P, free], F16, tag="q", name=f"q{name}{g}")
            nc.vector.tensor_scalar(q, xg, float(g + 1), 0.0, OP.subtract, OP.max)
            p2 = tmp.tile([P, free], F16, tag="p2", name=f"p2{name}{g}")
            nc.scalar.activation(p2, p, AF.Square)
            q24 = tmp.tile([P, free], F16, tag="q24", name=f"q24{name}{g}")
            nc.scalar.activation(q24, q, AF.Square, scale=2.0)  # (2q)^2 = 4q^2
            p3 = tmp.tile([P, free], F16, tag="p3", bufs=2, name=f"p3{name}{g}")
            nc.vector.tensor_mul(p3, p2, p)
            q34 = tmp.tile([P, free], F16, tag="q34", bufs=2, name=f"q34{name}{g}")
            nc.vector.tensor_mul(q34, q24, q)
            bt = btp.tile([P, free], F16, tag="bt", name=f"bt{name}{g}")
            nc.vector.tensor_sub(bt, q34, p3)
            return bt

        for ib in range(4):  # silu base path
            l1_block(ib, si1[:, ib * TOK : (ib + 1) * TOK])
        for g in range(NB):
            bt = emit_bspline(g, u1, r1, F1, tmp1, bt1, "a")
            for ib in range(4):
                l1_block(4 + g * 4 + ib, bt[:, ib * TOK : (ib + 1) * TOK])

        # ---- boundary: h in PSUM [h_blk, tok]; derive L2 u/silu -------
        u2 = persist.tile([P, F2], F16, tag="u2", name="u2")
        si2 = persist.tile([P, F2], F16, tag="si2", name="si2")
        for j in range(8):
            nc.scalar.activation(
                u2[:, j * TOK : (j + 1) * TOK], pb[j], AF.Relu, bias=bias_ap(5.5), scale=2.5
            )
            nc.scalar.activation(si2[:, j * TOK : (j + 1) * TOK], pb[j], AF.Silu)
        r2 = persist.tile([P, F2], F16, tag="r2", name="r2")
        nc.vector.tensor_scalar_mul(r2, u2, -1.0)

        # ---- L2 matmuls: out2[tok_blk, o] over 72 K-tiles -------------
        qb = [psum.tile([P, O], F32, tag=f"p{tb}", name=f"q{tb}") for tb in range(4)]

        def l2_block(k, lhsT_tile):
            wt = w2p.tile([P, O], F16, tag="w2k", name=f"w2k{k}")
            nc.sync.dma_start(out=wt, in_=w2[k])
            last = k == L2_NK - 1
            for tb in range(4):
                nc.tensor.matmul(
                    qb[tb],
                    lhsT_tile[:, tb * P : (tb + 1) * P],
                    wt,
                    start=(k == 0),
                    stop=last,
                )

        for j in range(8):  # silu base path
            l2_block(j, si2[:, j * TOK : (j + 1) * TOK])
        HF = F2 // 2  # L2 features built in two halves (SBUF pressure)
        for hf in range(2):
            sl = slice(hf * HF, (hf + 1) * HF)
            for g in range(NB):
                bt = emit_bspline(g, u2[:, sl], r2[:, sl], HF, tmp2, bt2, f"b{hf}")
                for jj in range(4):
                    j = hf * 4 + jj
                    l2_block(8 + g * 8 + j, bt[:, jj * TOK : (jj + 1) * TOK])

        # ---- store ----------------------------------------------------
        for tb in range(4):
            ot = outp.tile([P, O], F32, tag="ot", name=f"ot{tb}")
            nc.vector.tensor_copy(ot, qb[tb])
            nc.sync.dma_start(out=out[tb * P : (tb + 1) * P, :], in_=ot)

    nc.compile()
    _BUILD_CACHE["nc"] = nc
    return nc


# ---------------------------------------------------------------- entry
def kernel(x, coef1, scale_base1, scale_sp1, coef2, scale_base2, scale_sp2,
           _want_trace=False):
    from concourse.bass_utils import run_bass_kernel_spmd

    x_flat = np.asarray(x, np.float32).reshape(NTOK, D)
    w1 = _pack_w1(np.asarray(coef1), np.asarray(scale_sp1), np.asarray(scale_base1))
    w2 = _pack_w2(np.asarray(coef2), np.asarray(scale_sp2), np.asarray(scale_base2))

    nc = _build_kernel()

    in_maps = []
    for c in range(NCORES):
        xs = x_flat[c * TOK : (c + 1) * TOK]  # (TOK, D)
        in_maps.append(
            {
                "xT": np.ascontiguousarray(xs.T),
                "w1": w1,
                "w2": w2,
            }
        )

    res = run_bass_kernel_spmd(
        nc, in_maps, core_ids=list(range(NCORES)), trace=_want_trace
    )
    outs = [res.results[c]["out"] for c in range(NCORES)]
    full = np.concatenate(outs, axis=0).reshape(x.shape[0], x.shape[1], O)
    if _want_trace:
        kernel._last_results = res  # stash for test harness profiling
    return full.astype(np.float32)


# revision 14
# speedup vs baseline: 3.2588x; 1.0059x over previous
"""KAN feed-forward on Trainium2 — Bass/Tile kernel, 8-core data-parallel.

Math transform: each KAN layer is
    y = silu(x) @ scale_base + einsum('nig,iog,io->no', B(x), coef, scale_sp)
with B the (G=5, K=3) uniform-grid B-spline basis: B_g(x) = b3(u - g),
u = 2.5 x + 5.5, g = 0..7, b3 the cardinal cubic B-spline on [0, 4].

Unlike the truncated-power expansion (features up to 11^3 = 1331 with massive
cancellation against the output scale ~12, which needs fp32 matmuls), the
B-spline basis is evaluated DIRECTLY on ACT/DVE in bounded arithmetic:
    x_g = min(u, 2g+4-u)        # tent coordinate; <0 outside support
    p   = relu(x_g - g)         # in [0, 2]
    q   = relu(x_g - g - 1)     # in [0, 1]
    Bt  = 4 q^3 - p^3 = -6 b3(u - g)   # in [-4, 0]
All intermediates are O(1) so fp16 feature/weight matmuls are accurate
(rel err ~2e-3 end to end), running the PE at full rate (4x over fp32),
with only 9 K-feature-groups (silu + 8 B) instead of 12.

Per-core layout (512 tokens/core, data-parallel over tokens):
  L1: out1[o_blk, tok] accumulated over 36 K-tiles (4 silu + 8 g x 4 iblk).
      Elementwise feature build batched at free-dim 2048 (all 4 iblks).
  L2: out2[tok_blk, o] over 72 K-tiles (8 silu + 8 g x 8 jblk); features
      batched at free-dim 4096.  L1's PSUM output [h, tok] is exactly the
      layout L2's feature construction needs — no transposes anywhere.
"""

import sys
from contextlib import ExitStack

import numpy as np

for _p in ("/opt/trn_rl_repo",):
    if _p not in sys.path:
        sys.path.insert(0, _p)

# ---------------------------------------------------------------- constants
NB = 8  # B-spline basis functions per input dim
D, H, O = 512, 1024, 512
NCORES = 8
NTOK = 4096
TOK = NTOK // NCORES  # 512 tokens per core
P = 128

L1_NK = 4 + NB * 4  # 36 K-tiles of 128
L2_NK = 8 + NB * 8  # 72 K-tiles of 128

_BUILD_CACHE: dict = {}


# ---------------------------------------------------------------- host prep
def _pack_w1(coef1, scale_sp1, scale_base1) -> np.ndarray:
    """-> (36, 128, 1024) f16: K-tile rows x hidden cols."""
    Wg = -(coef1.astype(np.float64) * scale_sp1.astype(np.float64)[:, :, None]) / 6.0
    w1 = np.empty((L1_NK, P, H), np.float16)
    for ib in range(4):
        w1[ib] = scale_base1[ib * P : (ib + 1) * P]
    for g in range(NB):
        for ib in range(4):
            w1[4 + g * 4 + ib] = Wg[ib * P : (ib + 1) * P, :, g]
    return np.ascontiguousarray(w1)


def _pack_w2(coef2, scale_sp2, scale_base2) -> np.ndarray:
    """-> (72, 128, 512) f16: K-tile rows x output cols."""
    Wg = -(coef2.astype(np.float64) * scale_sp2.astype(np.float64)[:, :, None]) / 6.0
    w2 = np.empty((L2_NK, P, O), np.float16)
    for j in range(8):
        w2[j] = scale_base2[j * P : (j + 1) * P]
    for g in range(NB):
        for j in range(8):
            w2[8 + g * 8 + j] = Wg[j * P : (j + 1) * P, :, g]
    return np.ascontiguousarray(w2)


# ---------------------------------------------------------------- bass build
def _build_kernel():
    """Build + compile the Bass program once; cached per process."""
    if "nc" in _BUILD_CACHE:
        return _BUILD_CACHE["nc"]

    import concourse.mybir as mybir
    import concourse.tile as tile
    from concourse import bacc

    AF = mybir.ActivationFunctionType
    OP = mybir.AluOpType
    F32 = mybir.dt.float32
    F16 = mybir.dt.float16

    nc = bacc.Bacc("TRN2", target_bir_lowering=False, debug=False, num_devices=NCORES)

    xT = nc.dram_tensor("xT", (D, TOK), F32, kind="ExternalInput").ap()
    w1 = nc.dram_tensor("w1", (L1_NK, P, H), F16, kind="ExternalInput").ap()
    w2 = nc.dram_tensor("w2", (L2_NK, P, O), F16, kind="ExternalInput").ap()
    out = nc.dram_tensor("out", (TOK, O), F32, kind="ExternalOutput").ap()

    with tile.TileContext(nc) as tc, ExitStack() as ctx:
        persist = ctx.enter_context(tc.tile_pool(name="persist", bufs=1))
        tmp1 = ctx.enter_context(tc.tile_pool(name="tmp1", bufs=2))
        bt1 = ctx.enter_context(tc.tile_pool(name="bt1", bufs=3))
        tmp2 = ctx.enter_context(tc.tile_pool(name="tmp2", bufs=2))
        bt2 = ctx.enter_context(tc.tile_pool(name="bt2", bufs=3))
        w1p = ctx.enter_context(tc.tile_pool(name="w1p", bufs=4))
        w2p = ctx.enter_context(tc.tile_pool(name="w2p", bufs=5))
        outp = ctx.enter_context(tc.tile_pool(name="outp", bufs=2))
        psum = ctx.enter_context(tc.tile_pool(name="psum", bufs=1, space="PSUM"))

        F1 = 4 * TOK   # L1 elementwise free dim (4 i-blocks batched)
        F2 = 8 * TOK   # L2 elementwise free dim (8 h-blocks batched)

        _bias_cache: dict = {}

        def bias_ap(val: float):
            if val not in _bias_cache:
                t = persist.tile([P, 1], F32, tag=f"bias{len(_bias_cache)}",
                                 name=f"bias_{len(_bias_cache)}")
                nc.vector.memset(t, val)
                _bias_cache[val] = t
            return _bias_cache[val]

        # ---- L1 prep: x -> u, silu -----------------------------------
        xb = persist.tile([P, F1], F32, tag="xb", name="xb")
        for ib in range(4):
            nc.sync.dma_start(
                out=xb[:, ib * TOK : (ib + 1) * TOK],
                in_=xT[ib * P : (ib + 1) * P, :],
            )
        # u = relu(2.5x + 5.5): clamp-at-0 is exact (all basis funcs vanish)
        u1 = persist.tile([P, F1], F16, tag="u1", name="u1")
        nc.scalar.activation(u1, xb, AF.Relu, bias=bias_ap(5.5), scale=2.5)
        r1 = persist.tile([P, F1], F16, tag="r1", name="r1")
        nc.vector.tensor_scalar_mul(r1, u1, -1.0)
        si1 = persist.tile([P, F1], F16, tag="si1", name="si1")
        nc.scalar.activation(si1, xb, AF.Silu)

        # ---- L1 matmuls: out1[o_blk, tok] over 36 K-tiles -------------
        pb = [psum.tile([P, TOK], F32, tag=f"p{ob}", name=f"p{ob}") for ob in range(8)]

        def l1_block(k, rhs_ap):
            wt = w1p.tile([P, H], F16, tag="w1k", name=f"w1k{k}")
            nc.sync.dma_start(out=wt, in_=w1[k])
            last = k == L1_NK - 1
            for ob in range(8):
                nc.tensor.matmul(
                    pb[ob],
                    wt[:, ob * P : (ob + 1) * P],
                    rhs_ap,
                    start=(k == 0),
                    stop=last,
                )

        def emit_bspline(g, u, r, free, tmp, btp, name, q_on_act=False):
            """Bt_g = 4q^3 - p^3 = -6*b3(u-g), all f16, bounded."""
            xg = tmp.tile([P, free], F16, tag="xg", name=f"xg{name}{g}")
            # xg = min(2g+4 - u, u)  (tent coordinate, fused)
            nc.vector.scalar_tensor_tensor(xg, r, float(2 * g + 4), u, OP.add, OP.min)
            p = tmp.tile([P, free], F16, tag="p", name=f"p{name}{g}")
            nc.vector.tensor_scalar(p, xg, float(g), 0.0, OP.subtract, OP.max)
            q = tmp.tile([P, free], F16, tag="q", name=f"q{name}{g}")
            if q_on_act:
                nc.scalar.activation(q, xg, AF.Relu, bias=bias_ap(float(-(g + 1))))
            else:
                nc.vector.tensor_scalar(q, xg, float(g + 1), 0.0, OP.subtract, OP.max)
            p2 = tmp.tile([P, free], F16, tag="p2", name=f"p2{name}{g}")
            nc.scalar.activation(p2, p, AF.Square)
            q24 = tmp.tile([P, free], F16, tag="q24", name=f"q24{name}{g}")
            nc.scalar.activation(q24, q, AF.Square, scale=2.0)  # (2q)^2 = 4q^2
            p3 = tmp.tile([P, free], F16, tag="p3", bufs=2, name=f"p3{name}{g}")
            nc.vector.tensor_mul(p3, p2, p)
            q34 = tmp.tile([P, free], F16, tag="q34", bufs=2, name=f"q34{name}{g}")
            nc.vector.tensor_mul(q34, q24, q)
            bt = btp.tile([P, free], F16, tag="bt", name=f"bt{name}{g}")
            nc.vector.tensor_sub(bt, q34, p3)
            return bt

        for ib in range(4):  # silu base path
            l1_block(ib, si1[:, ib * TOK : (ib + 1) * TOK])
        for g in range(NB):
            bt = emit_bspline(g, u1, r1, F1, tmp1, bt1, "a")
            for ib in range(4):
                l1_block(4 + g * 4 + ib, bt[:, ib * TOK : (ib + 1) * TOK])

        # ---- boundary: h in PSUM [h_blk, tok]; derive L2 u/silu -------
        u2 = persist.tile([P, F2], F16, tag="u2", name="u2")
        si2 = persist.tile([P, F2], F16, tag="si2", name="si2")
        for j in range(8):
            nc.scalar.activation(
                u2[:, j * TOK : (j + 1) * TOK], pb[j], AF.Relu, bias=bias_ap(5.5), scale=2.5
            )
            nc.scalar.activation(si2[:, j * TOK : (j + 1) * TOK], pb[j], AF.Silu)
        r2 = persist.tile([P, F2], F16, tag="r2", name="r2")
        nc.vector.tensor_scalar_mul(r2, u2, -1.0)

        # ---- L2 matmuls: out2[tok_blk, o] over 72 K-tiles -------------
        qb = [psum.tile([P, O], F32, tag=f"p{tb}", name=f"q{tb}") for tb in range(4)]

        def l2_block(k, lhsT_tile):
            wt = w2p.tile([P, O], F16, tag="w2k", name=f"w2k{k}")
            nc.sync.dma_start(out=wt, in_=w2[k])
            last = k == L2_NK - 1
            for tb in range(4):
                nc.tensor.matmul(
                    qb[tb],
                    lhsT_tile[:, tb * P : (tb + 1) * P],
                    wt,
                    start=(k == 0),
                    stop=last,
                )

        for j in range(8):  # silu base path
            l2_block(j, si2[:, j * TOK : (j + 1) * TOK])
        HF = F2 // 2  # L2 features built in two halves (SBUF pressure)
        for hf in range(2):
            sl = slice(hf * HF, (hf + 1) * HF)
            for g in range(NB):
                bt = emit_bspline(g, u2[:, sl], r2[:, sl], HF, tmp2, bt2, f"b{hf}", q_on_act=True)
                for jj in range(4):
                    j = hf * 4 + jj
                    l2_block(8 + g * 8 + j, bt[:, jj * TOK : (jj + 1) * TOK])

        # ---- store ----------------------------------------------------
        for tb in range(4):
            ot = outp.tile([P, O], F32, tag="ot", name=f"ot{tb}")
            nc.scalar.activation(ot, qb[tb], AF.Identity)
            nc.sync.dma_start(out=out[tb * P : (tb + 1) * P, :], in_=ot)

    nc.compile()
    _BUILD_CACHE["nc"] = nc
    return nc


# ---------------------------------------------------------------- entry
def kernel(x, coef1, scale_base1, scale_sp1, coef2, scale_base2, scale_sp2,
           _want_trace=False):
    from concourse.bass_utils import run_bass_kernel_spmd

    x_flat = np.asarray(x, np.float32).reshape(NTOK, D)
    w1 = _pack_w1(np.asarray(coef1), np.asarray(scale_sp1), np.asarray(scale_base1))
    w2 = _pack_w2(np.asarray(coef2), np.asarray(scale_sp2), np.asarray(scale_base2))

    nc = _build_kernel()

    in_maps = []
    for c in range(NCORES):
        xs = x_flat[c * TOK : (c + 1) * TOK]  # (TOK, D)
        in_maps.append(
            {
                "xT": np.ascontiguousarray(xs.T),
                "w1": w1,
                "w2": w2,
            }
        )

    res = run_bass_kernel_spmd(
        nc, in_maps, core_ids=list(range(NCORES)), trace=_want_trace
    )
    outs = [res.results[c]["out"] for c in range(NCORES)]
    full = np.concatenate(outs, axis=0).reshape(x.shape[0], x.shape[1], O)
    if _want_trace:
        kernel._last_results = res  # stash for test harness profiling
    return full.astype(np.float32)


# revision 16
# speedup vs baseline: 3.2844x; 1.0078x over previous
"""KAN feed-forward on Trainium2 — Bass/Tile kernel, 8-core data-parallel.

Math transform: each KAN layer is
    y = silu(x) @ scale_base + einsum('nig,iog,io->no', B(x), coef, scale_sp)
with B the (G=5, K=3) uniform-grid B-spline basis: B_g(x) = b3(u - g),
u = 2.5 x + 5.5, g = 0..7, b3 the cardinal cubic B-spline on [0, 4].

Unlike the truncated-power expansion (features up to 11^3 = 1331 with massive
cancellation against the output scale ~12, which needs fp32 matmuls), the
B-spline basis is evaluated DIRECTLY on ACT/DVE in bounded arithmetic:
    x_g = min(u, 2g+4-u)        # tent coordinate; <0 outside support
    p   = relu(x_g - g)         # in [0, 2]
    q   = relu(x_g - g - 1)     # in [0, 1]
    Bt  = 4 q^3 - p^3 = -6 b3(u - g)   # in [-4, 0]
All intermediates are O(1) so fp16 feature/weight matmuls are accurate
(rel err ~2e-3 end to end), running the PE at full rate (4x over fp32),
with only 9 K-feature-groups (silu + 8 B) instead of 12.

Per-core layout (512 tokens/core, data-parallel over tokens):
  L1: out1[o_blk, tok] accumulated over 36 K-tiles (4 silu + 8 g x 4 iblk).
      Elementwise feature build batched at free-dim 2048 (all 4 iblks).
  L2: out2[tok_blk, o] over 72 K-tiles (8 silu + 8 g x 8 jblk); features
      batched at free-dim 4096.  L1's PSUM output [h, tok] is exactly the
      layout L2's feature construction needs — no transposes anywhere.
"""

import sys
from contextlib import ExitStack

import numpy as np

for _p in ("/opt/trn_rl_repo",):
    if _p not in sys.path:
        sys.path.insert(0, _p)

# ---------------------------------------------------------------- constants
NB = 8  # B-spline basis functions per input dim
D, H, O = 512, 1024, 512
NCORES = 8
NTOK = 4096
TOK = NTOK // NCORES  # 512 tokens per core
P = 128

L1_NK = 4 + NB * 4  # 36 K-tiles of 128
L2_NK = 8 + NB * 8  # 72 K-tiles of 128

_BUILD_CACHE: dict = {}


# ---------------------------------------------------------------- host prep
def _pack_w1(coef1, scale_sp1, scale_base1) -> np.ndarray:
    """-> (36, 128, 1024) f16: K-tile rows x hidden cols."""
    Wg = -(coef1.astype(np.float64) * scale_sp1.astype(np.float64)[:, :, None]) / 6.0
    w1 = np.empty((L1_NK, P, H), np.float16)
    for ib in range(4):
        w1[ib] = scale_base1[ib * P : (ib + 1) * P]
    for g in range(NB):
        for ib in range(4):
            w1[4 + g * 4 + ib] = Wg[ib * P : (ib + 1) * P, :, g]
    return np.ascontiguousarray(w1)


def _pack_w2(coef2, scale_sp2, scale_base2) -> np.ndarray:
    """-> (72, 128, 512) f16: K-tile rows x output cols."""
    Wg = -(coef2.astype(np.float64) * scale_sp2.astype(np.float64)[:, :, None]) / 6.0
    w2 = np.empty((L2_NK, P, O), np.float16)
    for j in range(8):
        w2[j] = scale_base2[j * P : (j + 1) * P]
    for g in range(NB):
        for j in range(8):
            w2[8 + g * 8 + j] = Wg[j * P : (j + 1) * P, :, g]
    return np.ascontiguousarray(w2)


# ---------------------------------------------------------------- bass build
def _build_kernel():
    """Build + compile the Bass program once; cached per process."""
    if "nc" in _BUILD_CACHE:
        return _BUILD_CACHE["nc"]

    import concourse.mybir as mybir
    import concourse.tile as tile
    from concourse import bacc

    AF = mybir.ActivationFunctionType
    OP = mybir.AluOpType
    F32 = mybir.dt.float32
    F16 = mybir.dt.float16

    nc = bacc.Bacc("TRN2", target_bir_lowering=False, debug=False, num_devices=NCORES)

    xT = nc.dram_tensor("xT", (D, TOK), F32, kind="ExternalInput").ap()
    w1 = nc.dram_tensor("w1", (L1_NK, P, H), F16, kind="ExternalInput").ap()
    w2 = nc.dram_tensor("w2", (L2_NK, P, O), F16, kind="ExternalInput").ap()
    out = nc.dram_tensor("out", (TOK, O), F32, kind="ExternalOutput").ap()

    with tile.TileContext(nc) as tc, ExitStack() as ctx:
        persist = ctx.enter_context(tc.tile_pool(name="persist", bufs=1))
        tmp1 = ctx.enter_context(tc.tile_pool(name="tmp1", bufs=2))
        bt1 = ctx.enter_context(tc.tile_pool(name="bt1", bufs=3))
        tmp2 = ctx.enter_context(tc.tile_pool(name="tmp2", bufs=2))
        bt2 = ctx.enter_context(tc.tile_pool(name="bt2", bufs=3))
        w1p = ctx.enter_context(tc.tile_pool(name="w1p", bufs=4))
        w2p = ctx.enter_context(tc.tile_pool(name="w2p", bufs=5))
        outp = ctx.enter_context(tc.tile_pool(name="outp", bufs=2))
        psum = ctx.enter_context(tc.tile_pool(name="psum", bufs=1, space="PSUM"))

        F1 = 4 * TOK   # L1 elementwise free dim (4 i-blocks batched)
        F2 = 8 * TOK   # L2 elementwise free dim (8 h-blocks batched)

        _bias_cache: dict = {}

        def bias_ap(val: float):
            if val not in _bias_cache:
                t = persist.tile([P, 1], F32, tag=f"bias{len(_bias_cache)}",
                                 name=f"bias_{len(_bias_cache)}")
                nc.vector.memset(t, val)
                _bias_cache[val] = t
            return _bias_cache[val]

        # ---- L1 prep: x -> u, silu -----------------------------------
        xb = persist.tile([P, F1], F32, tag="xb", name="xb")
        for ib in range(4):
            nc.sync.dma_start(
                out=xb[:, ib * TOK : (ib + 1) * TOK],
                in_=xT[ib * P : (ib + 1) * P, :],
            )
        # u = relu(2.5x + 5.5): clamp-at-0 is exact (all basis funcs vanish)
        u1 = persist.tile([P, F1], F16, tag="u1", name="u1")
        nc.scalar.activation(u1, xb, AF.Relu, bias=bias_ap(5.5), scale=2.5)
        r1 = persist.tile([P, F1], F16, tag="r1", name="r1")
        nc.vector.tensor_scalar_mul(r1, u1, -1.0)
        si1 = persist.tile([P, F1], F16, tag="si1", name="si1")
        nc.scalar.activation(si1, xb, AF.Silu)

        # ---- L1 matmuls: out1[o_blk, tok] over 36 K-tiles -------------
        pb = [psum.tile([P, TOK], F32, tag=f"p{ob}", name=f"p{ob}") for ob in range(8)]

        def l1_block(k, rhs_ap):
            wt = w1p.tile([P, H], F16, tag="w1k", name=f"w1k{k}")
            nc.sync.dma_start(out=wt, in_=w1[k])
            last = k == L1_NK - 1
            for ob in range(8):
                nc.tensor.matmul(
                    pb[ob],
                    wt[:, ob * P : (ob + 1) * P],
                    rhs_ap,
                    start=(k == 0),
                    stop=last,
                )

        def emit_bspline(g, u, r, free, tmp, btp, name, q_on_act=False):
            """Bt_g = 4q^3 - p^3 = -6*b3(u-g), all f16, bounded."""
            xg = tmp.tile([P, free], F16, tag="xg", name=f"xg{name}{g}")
            # xg = min(2g+4 - u, u)  (tent coordinate, fused)
            nc.vector.scalar_tensor_tensor(xg, r, float(2 * g + 4), u, OP.add, OP.min)
            p = tmp.tile([P, free], F16, tag="p", name=f"p{name}{g}")
            nc.vector.tensor_scalar(p, xg, float(g), 0.0, OP.subtract, OP.max)
            q = tmp.tile([P, free], F16, tag="q", name=f"q{name}{g}")
            if q_on_act:
                nc.scalar.activation(q, xg, AF.Relu, bias=bias_ap(float(-(g + 1))))
            else:
                nc.vector.tensor_scalar(q, xg, float(g + 1), 0.0, OP.subtract, OP.max)
            p2 = tmp.tile([P, free], F16, tag="p2", name=f"p2{name}{g}")
            nc.scalar.activation(p2, p, AF.Square)
            q24 = tmp.tile([P, free], F16, tag="q24", name=f"q24{name}{g}")
            nc.scalar.activation(q24, q, AF.Square, scale=2.0)  # (2q)^2 = 4q^2
            p3 = tmp.tile([P, free], F16, tag="p3", bufs=2, name=f"p3{name}{g}")
            nc.vector.tensor_mul(p3, p2, p)
            q34 = tmp.tile([P, free], F16, tag="q34", bufs=2, name=f"q34{name}{g}")
            nc.vector.tensor_mul(q34, q24, q)
            bt = btp.tile([P, free], F16, tag="bt", name=f"bt{name}{g}")
            nc.vector.tensor_sub(bt, q34, p3)
            return bt

        for ib in range(4):  # silu base path
            l1_block(ib, si1[:, ib * TOK : (ib + 1) * TOK])
        for g in range(NB):
            bt = emit_bspline(g, u1, r1, F1, tmp1, bt1, "a")
            for ib in range(4):
                l1_block(4 + g * 4 + ib, bt[:, ib * TOK : (ib + 1) * TOK])

        # ---- boundary: h in PSUM [h_blk, tok]; derive L2 u/silu -------
        u2 = persist.tile([P, F2], F16, tag="u2", name="u2")
        si2 = persist.tile([P, F2], F16, tag="si2", name="si2")
        for j in range(8):
            nc.scalar.activation(
                u2[:, j * TOK : (j + 1) * TOK], pb[j], AF.Relu, bias=bias_ap(5.5), scale=2.5
            )
            nc.scalar.activation(si2[:, j * TOK : (j + 1) * TOK], pb[j], AF.Silu)
        r2 = persist.tile([P, F2], F16, tag="r2", name="r2")
        nc.vector.tensor_scalar_mul(r2, u2, -1.0)

        # ---- L2 matmuls: out2[tok_blk, o] over 72 K-tiles -------------
        qb = [psum.tile([P, O], F32, tag=f"p{tb}", name=f"q{tb}") for tb in range(4)]

        def l2_block(k, lhsT_tile):
            wt = w2p.tile([P, O], F16, tag="w2k", name=f"w2k{k}")
            nc.sync.dma_start(out=wt, in_=w2[k])
            last = k == L2_NK - 1
            for tb in range(4):
                nc.tensor.matmul(
                    qb[tb],
                    lhsT_tile[:, tb * P : (tb + 1) * P],
                    wt,
                    start=(k == 0),
                    stop=last,
                )

        for j in range(8):  # silu base path
            l2_block(j, si2[:, j * TOK : (j + 1) * TOK])
        HF = F2 // 2  # L2 features built in two halves (SBUF pressure)
        for hf in range(2):
            sl = slice(hf * HF, (hf + 1) * HF)
            for g in range(NB):
                bt = emit_bspline(g, u2[:, sl], r2[:, sl], HF, tmp2, bt2, f"b{hf}", q_on_act=True)
                for jj in range(4):
                    j = hf * 4 + jj
                    l2_block(8 + g * 8 + j, bt[:, jj * TOK : (jj + 1) * TOK])

        # ---- store ----------------------------------------------------
        for tb in range(4):
            ot = outp.tile([P, O], F32, tag="ot", name=f"ot{tb}")
            nc.scalar.activation(ot, qb[tb], AF.Identity)
            nc.sync.dma_start(out=out[tb * P : (tb + 1) * P, :], in_=ot)

    nc.compile()
    _BUILD_CACHE["nc"] = nc
    return nc


# ---------------------------------------------------------------- entry
def kernel(x, coef1, scale_base1, scale_sp1, coef2, scale_base2, scale_sp2,
           _want_trace=False):
    from concourse.bass_utils import run_bass_kernel_spmd

    x_flat = np.asarray(x, np.float32).reshape(NTOK, D)
    w1 = _pack_w1(np.asarray(coef1), np.asarray(scale_sp1), np.asarray(scale_base1))
    w2 = _pack_w2(np.asarray(coef2), np.asarray(scale_sp2), np.asarray(scale_base2))

    nc = _build_kernel()

    in_maps = []
    for c in range(NCORES):
        xs = x_flat[c * TOK : (c + 1) * TOK]  # (TOK, D)
        in_maps.append(
            {
                "xT": np.ascontiguousarray(xs.T),
                "w1": w1,
                "w2": w2,
            }
        )

    res = run_bass_kernel_spmd(
        nc, in_maps, core_ids=list(range(NCORES)), trace=_want_trace
    )
    outs = [res.results[c]["out"] for c in range(NCORES)]
    full = np.concatenate(outs, axis=0).reshape(x.shape[0], x.shape[1], O)
    if _want_trace:
        kernel._last_results = res  # stash for test harness profiling
    return full.astype(np.float32)
